# revision 27
# baseline (speedup 1.0000x reference)
"""Trainium2 Bass kernel for the hierarchical GNN message-passing block.

Math (per sample n):
  x_t = max_T x                                  [C, L, V]
  h   = relu(BNd(Wd @ x_t))                      [C4, L, V]
  s_l = mean_{v in LAYERS[l]} h[:, l, v]         [C4, L]
  EdgeConv on the L=6 node graph with kNN (K=3):
     dist'[l, j] = 2 * s_l . s_j - |s_j|^2       (row-constant dropped; same top-k)
     nbr(l) = top-3 of row l (found via 3rd-largest threshold, no argmax)
     z[c,l,k=j] = A[c,j] + B[c,l],  A = W1' s, B = (W2'-W1') s  (BN scale folded,
        positive scale + monotone leaky-relu commute with the max over k)
     e = leaky(max_{j in nbr(l)} A[c,j] + B[c,l] + bias_e)
  gate = sigmoid(W_agg e + b_agg)                [C, L]
  out  = sum_l gate[:, l] * x[:, l]              [C, T, V]

Mapping: batch N=32 data-parallel over 8 cores (4 samples each). Per core:
x[n] is read from HBM exactly once, cast f32->bf16 in the SWDGE DMA, and
stays resident in SBUF (bf16 keeps 3 samples in flight); max-over-T is a
binary tree of contiguous bf16 tensor_tensor max ops on DVE (2x packed
mode); the tiny graph chain runs on PE/DVE/ACT with small matmuls (incl.
one-hot selector matmuls for partition broadcasts of the neighbor mask);
the final gated sum runs on the TensorEngine as diag(gate) bf16 matmuls
accumulating over L in fp32 PSUM. Loads own the SWDGE ring; stores
alternate across the two HWDGE rings. Measured ~160-180us on silicon
(HBM-read roofline for the 39.3MB/core input is ~110us), rel err ~3.5e-3
(bf16 rounding; tolerance 2e-2).
"""

import sys

import numpy as np

for _p in ("/opt/trn_rl_repo", "/root/.axon_site/_ro/trn_rl_repo"):
    if _p not in sys.path:
        sys.path.append(_p)

N, C, L, T, V = 32, 256, 6, 64, 25
C4 = C // 4
NCORES = 8
NLOC = N // NCORES
EPS = 1e-5
SLOPE = 0.2
BIG = 1.0e30
TV = T * V
CH = [(0, 512), (512, 512), (1024, 512), (1536, TV - 1536)]

_G = [[1], [0, 20], [12, 16, 2, 4, 8], [13, 17, 3, 5, 9], [14, 18, 6, 10],
      [15, 19, 7, 11], [21, 22, 23, 24]]
LAYERS = [sorted(_G[i] + _G[i + 1]) for i in range(L)]

# Each layer's joint subset as exactly two affine access patterns
# (start, [[step, count], ...]) over the V axis; verified in _check_aps().
LAYER_APS = [
    ((0, [[1, 2]]), (20, [[1, 1]])),                 # {0,1} + {20}
    ((0, [[2, 3]]), (8, [[4, 4]])),                  # {0,2,4} + {8,12,16,20}
    ((2, [[1, 4]]), (8, [[4, 3], [1, 2]])),          # {2..5} + {8,9,12,13,16,17}
    ((3, [[1, 1]]), (5, [[4, 4], [1, 2]])),          # {3} + {5,6,9,10,13,14,17,18}
    ((6, [[4, 2], [1, 2]]), (14, [[4, 2], [1, 2]])),  # {6,7,10,11} + {14,15,18,19}
    ((7, [[4, 4]]), (21, [[1, 4]])),                 # {7,11,15,19} + {21..24}
]


def _ap_indices(start, steps):
    idx = [start]
    for step, count in steps:
        idx = [i + step * k for i in idx for k in range(count)]
    return sorted(idx)


def _check_aps():
    for l in range(L):
        (s0, a0), (s1, a1) = LAYER_APS[l]
        got = sorted(_ap_indices(s0, a0) + _ap_indices(s1, a1))
        assert got == LAYERS[l], (l, got, LAYERS[l])


_check_aps()

_NC_CACHE = {}


def _build_nc():
    import concourse.bacc as bacc
    import concourse.bass as bass
    import concourse.tile as tile
    from concourse import mybir
    from contextlib import ExitStack

    f32 = mybir.dt.float32
    bf16 = mybir.dt.bfloat16
    AX = mybir.AxisListType
    OP = mybir.AluOpType
    AF = mybir.ActivationFunctionType

    # larger SWDGE descriptor ring: the bf16-cast x loads run on SWDGE and
    # the default 16KB ring forces a Q7 drain every ~2 transfers
    nc = bacc.Bacc(None, target_bir_lowering=False,
                   dynamic_dma_scratch_size=49152, num_swdge_queues=2)

    x_d = nc.declare_dram_parameter("x", [NLOC, C, L, T, V], f32, isOutput=False)
    wdt_d = nc.declare_dram_parameter("wd_t", [C, C4], f32, isOutput=False)
    w1t_d = nc.declare_dram_parameter("w1_t", [C4, C4], f32, isOutput=False)
    w21t_d = nc.declare_dram_parameter("w21_t", [C4, C4], f32, isOutput=False)
    wat_d = nc.declare_dram_parameter("wagg_t", [C4, C], f32, isOutput=False)
    sel_d = nc.declare_dram_parameter("sel", [L, L * C4], f32, isOutput=False)
    id_d = nc.declare_dram_parameter("ident", [128, 128], f32, isOutput=False)
    bd_d = nc.declare_dram_parameter("bias_d", [C4, 1], f32, isOutput=False)
    be_d = nc.declare_dram_parameter("bias_e", [C4, 1], f32, isOutput=False)
    bg_d = nc.declare_dram_parameter("bias_g", [128, 2], f32, isOutput=False)
    msk_d = nc.declare_dram_parameter("mask", [C4, L, V], f32, isOutput=False)
    out_d = nc.declare_dram_parameter("out", [NLOC, C, T, V], f32, isOutput=True)

    with tile.TileContext(nc) as tc, ExitStack() as ctx:
        const = ctx.enter_context(tc.tile_pool(name="const", bufs=1))
        xpool = ctx.enter_context(tc.tile_pool(name="xpool", bufs=6))
        mxpool = ctx.enter_context(tc.tile_pool(name="mxpool", bufs=1))
        sm = ctx.enter_context(tc.tile_pool(name="sm", bufs=3))
        dpool = ctx.enter_context(tc.tile_pool(name="dpool", bufs=3))
        opool = ctx.enter_context(tc.tile_pool(name="opool", bufs=3))
        ps = ctx.enter_context(
            tc.tile_pool(name="ps", bufs=4, space=bass.MemorySpace.PSUM))
        pso = ctx.enter_context(
            tc.tile_pool(name="pso", bufs=4, space=bass.MemorySpace.PSUM))

        # ---- constants into SBUF ----
        wdt_sb = const.tile([128, 2, C4], bf16, tag="wdt")
        nc.gpsimd.dma_start(out=wdt_sb, in_=wdt_d[:].rearrange("(k p) m -> p k m", p=128))
        w1t_sb = const.tile([C4, C4], f32, tag="w1t")
        nc.sync.dma_start(out=w1t_sb, in_=w1t_d[:])
        w21t_sb = const.tile([C4, C4], f32, tag="w21t")
        nc.sync.dma_start(out=w21t_sb, in_=w21t_d[:])
        wat_sb = const.tile([C4, 2, 128], f32, tag="wat")
        nc.sync.dma_start(out=wat_sb, in_=wat_d[:].rearrange("p (k m) -> p k m", k=2))
        sel_sb = const.tile([L, L * C4], f32, tag="sel")
        nc.sync.dma_start(out=sel_sb, in_=sel_d[:])
        id_sb = const.tile([128, 128], f32, tag="ident")
        nc.sync.dma_start(out=id_sb, in_=id_d[:])
        bd_sb = const.tile([C4, 1], f32, tag="bd")
        nc.sync.dma_start(out=bd_sb, in_=bd_d[:])
        be_sb = const.tile([C4, 1], f32, tag="be")
        nc.sync.dma_start(out=be_sb, in_=be_d[:])
        bg_sb = const.tile([128, 2], f32, tag="bg")
        nc.sync.dma_start(out=bg_sb, in_=bg_d[:])
        msk_sb = const.tile([C4, L, V], f32, tag="msk")
        nc.sync.dma_start(out=msk_sb, in_=msk_d[:])
        ones_sb = const.tile([C4, 8], f32, tag="ones")
        nc.vector.memset(ones_sb, 1.0)
        negb_sb = const.tile([L, L], f32, tag="negb")
        nc.vector.memset(negb_sb, -BIG)

        for n in range(NLOC):
            # ---- load x[n] (both channel halves), keep resident ----
            # x arrives in two T-halves per channel-half so the max tree can
            # start while the second half is still streaming from HBM
            xh = []
            for h in range(2):
                xt_ = xpool.tile([128, L, T, V], bf16, tag="x", name=f"x_{n}_{h}")
                nc.gpsimd.dma_start(out=xt_, in_=x_d[n, h * 128:(h + 1) * 128])
                xh.append(xt_)

            # ---- pass 1: max over T as a binary tree of contiguous
            # tensor_tensor max ops (a strided-inner reduce_max runs at
            # ~0.5 elem/cycle; contiguous TT max streams at 1x) ----
            xt = sm.tile([128, 2, L, V], bf16, tag="xt", name=f"xt_{n}")
            for h in range(2):
                xf = xh[h]
                ta = mxpool.tile([128, L, 32, V], bf16, tag="mxa", bufs=2,
                                 name=f"mxa_{n}{h}")
                nc.vector.tensor_max(ta[:], xf[:, :, 0:32], xf[:, :, 32:64])
                for d in (16, 8, 4, 2):
                    nc.vector.tensor_max(
                        ta[:, :, 0:d], ta[:, :, 0:d], ta[:, :, d:2 * d])
                nc.vector.tensor_max(
                    xt[:, h], ta[:, :, 0, :], ta[:, :, 1, :])

            # ---- conv_down + BN + relu -> h_sb [C4, L, V] ----
            ps_h = ps.tile([C4, L * V], f32, tag="ps", name=f"psh_{n}")
            for h in range(2):
                nc.tensor.matmul(
                    ps_h[:],
                    lhsT=wdt_sb[:, h],
                    rhs=xt[:, h].rearrange("p l v -> p (l v)"),
                    start=(h == 0),
                    stop=(h == 1),
                )
            # relu(conv + bias) fused on DVE: (x + bias) max 0
            h_sb = sm.tile([C4, L, V], f32, tag="h", name=f"h_{n}")
            nc.vector.tensor_scalar(
                out=h_sb.rearrange("p l v -> p (l v)"), in0=ps_h,
                scalar1=bd_sb[:, 0:1], scalar2=0.0, op0=OP.add, op1=OP.max)

            # ---- hierarchy sampling: s[c, l] = sum_v h[c, l, v] * mask[l, v]
            # (mask carries the subset indicator and the 1/k scale) ----
            hm = sm.tile([C4, L, V], f32, tag="hm", name=f"hm_{n}")
            nc.vector.tensor_mul(hm, h_sb, msk_sb)
            s_sb = sm.tile([C4, L], f32, tag="s", name=f"s_{n}")
            nc.vector.reduce_sum(out=s_sb, in_=hm, axis=AX.X)

            # ---- A = W1' s ; B = (W2'-W1') s ----
            ps_a = ps.tile([C4, L], f32, tag="ps", name=f"psa_{n}")
            nc.tensor.matmul(ps_a[:], lhsT=w1t_sb[:], rhs=s_sb[:], start=True, stop=True)
            ps_b = ps.tile([C4, L], f32, tag="ps", name=f"psb_{n}")
            nc.tensor.matmul(ps_b[:], lhsT=w21t_sb[:], rhs=s_sb[:], start=True, stop=True)
            a_sb = sm.tile([C4, L], f32, tag="a", name=f"a_{n}")
            nc.scalar.copy(a_sb, ps_a)

            # ---- kNN: dist'[l,j] = 2*inner[l,j] - sq[j] ----
            s2 = sm.tile([C4, L], f32, tag="s2", name=f"s2_{n}")
            nc.vector.tensor_mul(s2, s_sb, s_sb)
            ps_in = ps.tile([L, L], f32, tag="ps", name=f"psin_{n}")
            nc.tensor.matmul(ps_in[:], lhsT=s_sb[:], rhs=s_sb[:], start=True, stop=True)
            in_sb = sm.tile([L, L], f32, tag="insb", name=f"insb_{n}")
            nc.scalar.copy(in_sb, ps_in)
            ps_sq = ps.tile([1, L], f32, tag="ps", name=f"pssq_{n}")
            nc.tensor.matmul(ps_sq[:], lhsT=ones_sb[:, 0:1], rhs=s2[:], start=True, stop=True)
            sq_sb = sm.tile([1, L], f32, tag="sq", name=f"sq_{n}")
            nc.scalar.copy(sq_sb, ps_sq)
            ps_sqb = ps.tile([L, L], f32, tag="ps", name=f"pssqb_{n}")
            nc.tensor.matmul(ps_sqb[:], lhsT=ones_sb[0:1, 0:L], rhs=sq_sb[:], start=True, stop=True)
            dist = sm.tile([L, L], f32, tag="dist", name=f"dist_{n}")
            nc.vector.scalar_tensor_tensor(
                out=dist, in0=in_sb, scalar=2.0, in1=ps_sqb,
                op0=OP.mult, op1=OP.subtract)

            # ---- third-largest per row -> neighborhood mask (0 / -BIG) ----
            mx = sm.tile([L, 3], f32, tag="mx", name=f"mx_{n}")
            nc.vector.reduce_max(out=mx[:, 0:1], in_=dist, axis=AX.X)
            eq1 = sm.tile([L, L], f32, tag="eq", name=f"eq1_{n}")
            nc.vector.tensor_scalar(
                out=eq1, in0=dist, scalar1=mx[:, 0:1], scalar2=None, op0=OP.is_equal)
            d2 = sm.tile([L, L], f32, tag="dmask", name=f"d2_{n}")
            nc.vector.scalar_tensor_tensor(
                out=d2, in0=eq1, scalar=-BIG, in1=dist, op0=OP.mult, op1=OP.add)
            nc.vector.reduce_max(out=mx[:, 1:2], in_=d2, axis=AX.X)
            eq2 = sm.tile([L, L], f32, tag="eq", name=f"eq2_{n}")
            nc.vector.tensor_scalar(
                out=eq2, in0=d2, scalar1=mx[:, 1:2], scalar2=None, op0=OP.is_equal)
            d3 = sm.tile([L, L], f32, tag="dmask", name=f"d3_{n}")
            nc.vector.scalar_tensor_tensor(
                out=d3, in0=eq2, scalar=-BIG, in1=d2, op0=OP.mult, op1=OP.add)
            nc.vector.reduce_max(out=mx[:, 2:3], in_=d3, axis=AX.X)
            nbr = sm.tile([L, L], f32, tag="nbr", name=f"nbr_{n}")
            nc.vector.scalar_tensor_tensor(
                out=nbr, in0=dist, scalar=mx[:, 2:3], in1=negb_sb,
                op0=OP.is_lt, op1=OP.mult)

            # ---- M[c, l] = max_j (A[c, j] + nbrmask[l, j]) ----
            ps_all = ps.tile([C4, L, L], f32, tag="ps", name=f"psall_{n}")
            for l in range(L):
                nc.tensor.matmul(
                    ps_all[:, l], lhsT=sel_sb[:, l * C4:(l + 1) * C4], rhs=nbr[:],
                    start=True, stop=True)
            scr = sm.tile([C4, L, L], f32, tag="scr", name=f"scr_{n}")
            a_bcast = bass.AP(
                tensor=a_sb.tensor, offset=a_sb.offset,
                ap=[list(a_sb.ap[0]), [0, L], [1, L]])
            nc.vector.tensor_add(scr, a_bcast, ps_all)
            m_sb = sm.tile([C4, L], f32, tag="m", name=f"m_{n}")
            nc.vector.reduce_max(out=m_sb, in_=scr, axis=AX.X)

            # ---- e = leaky(B + bias_e + M) = max(zz, 0.2*zz) ----
            zz = sm.tile([C4, L], f32, tag="zz", name=f"zz_{n}")
            nc.vector.scalar_tensor_tensor(
                out=zz, in0=ps_b, scalar=be_sb[:, 0:1], in1=m_sb,
                op0=OP.add, op1=OP.add)
            e_sb = sm.tile([C4, L], f32, tag="e", name=f"e_{n}")
            nc.vector.scalar_tensor_tensor(
                out=e_sb, in0=zz, scalar=SLOPE, in1=zz, op0=OP.mult, op1=OP.max)

            # ---- gate = sigmoid(W_agg e + b_agg), per channel half ----
            gate = sm.tile([128, 2, L], f32, tag="gate", name=f"gate_{n}")
            for h in range(2):
                ps_at = ps.tile([128, L], f32, tag="ps", name=f"psat_{n}_{h}")
                nc.tensor.matmul(
                    ps_at[:], lhsT=wat_sb[:, h], rhs=e_sb[:], start=True, stop=True)
                nc.scalar.activation(
                    gate[:, h], ps_at, AF.Sigmoid, bias=bg_sb[:, h:h + 1])

            # ---- pass 2: out[c, tv] = sum_l gate[c, l] * x[c, l, tv] ----
            for h in range(2):
                diags = []
                for l in range(L):
                    dg = dpool.tile([128, 128], bf16, tag="diag", name=f"dg_{n}_{h}_{l}")
                    nc.vector.tensor_scalar(
                        out=dg, in0=id_sb, scalar1=gate[:, h, l:l + 1],
                        scalar2=None, op0=OP.mult)
                    diags.append(dg)
                xflat = xh[h].rearrange("p l t v -> p l (t v)")
                o_sb = opool.tile([128, TV], f32, tag="osb", name=f"o_{n}_{h}")
                for ci, (c0, w) in enumerate(CH):
                    ps_o = pso.tile([128, 512], f32, tag="pso", name=f"pso_{n}_{h}_{c0}")
                    for l in range(L):
                        nc.tensor.matmul(
                            ps_o[:, :w],
                            lhsT=diags[l],
                            rhs=xflat[:, l, c0:c0 + w],
                            start=(l == 0),
                            stop=(l == L - 1),
                        )
                    # alternate the PSUM drain between ACT and DVE so the
                    # tail isn't serialized on one engine
                    if ci % 2 == 0:
                        nc.scalar.copy(o_sb[:, c0:c0 + w], ps_o[:, :w])
                    else:
                        nc.vector.tensor_copy(o_sb[:, c0:c0 + w], ps_o[:, :w])
                # stores alternate between the two HWDGE rings (SP/ACT) so
                # consecutive stores don't serialize on one FIFO; loads own
                # the SWDGE ring
                eng = nc.sync if h == 0 else nc.scalar
                eng.dma_start(
                    out=out_d[n, h * 128:(h + 1) * 128].rearrange("p t v -> p (t v)"),
                    in_=o_sb)

    nc.compile()
    return nc


def _get_nc():
    if "nc" not in _NC_CACHE:
        _NC_CACHE["nc"] = _build_nc()
    return _NC_CACHE["nc"]


def _host_prep(inputs):
    f = np.float32
    g_down = inputs["g_down"].astype(f)
    v_down = inputs["v_down"].astype(f)
    m_down = inputs["m_down"].astype(f)
    be_down = inputs["be_down"].astype(f)
    b_down = inputs["b_down"].astype(f)
    W_down = inputs["W_down"].astype(f)
    sd = g_down / np.sqrt(v_down + EPS)
    wd_eff = W_down * sd[:, None]
    bias_d = ((b_down - m_down) * sd + be_down).reshape(C4, 1)

    g_e = inputs["g_edge"].astype(f)
    v_e = inputs["v_edge"].astype(f)
    m_e = inputs["m_edge"].astype(f)
    be_e = inputs["be_edge"].astype(f)
    W_edge = inputs["W_edge"].astype(f)
    se = g_e / np.sqrt(v_e + EPS)
    W1 = W_edge[:, :C4] * se[:, None]
    W2 = W_edge[:, C4:] * se[:, None]
    bias_e = (be_e - m_e * se).reshape(C4, 1)

    W_agg = inputs["W_agg"].astype(f)
    b_agg = inputs["b_agg"].astype(f)

    sel = np.zeros((L, L * C4), f)
    for l in range(L):
        sel[l, l * C4:(l + 1) * C4] = 1.0
    mask = np.zeros((L, V), f)
    for l in range(L):
        mask[l, LAYERS[l]] = 1.0 / len(LAYERS[l])
    mask = np.broadcast_to(mask[None], (C4, L, V))
    consts = {
        "wd_t": np.ascontiguousarray(wd_eff.T),
        "w1_t": np.ascontiguousarray(W1.T),
        "w21_t": np.ascontiguousarray((W2 - W1).T),
        "wagg_t": np.ascontiguousarray(W_agg.T),
        "sel": sel,
        "ident": np.eye(128, dtype=f),
        "bias_d": bias_d,
        "bias_e": bias_e,
        "bias_g": np.ascontiguousarray(b_agg.reshape(2, 128).T),
        "mask": np.ascontiguousarray(mask),
    }
    return consts


def _run(inputs, trace=False):
    import time

    from concourse.bass_utils import run_bass_kernel_spmd

    consts = _host_prep(inputs)
    x = np.asarray(inputs["x"], np.float32)
    in_maps = []
    for i in range(NCORES):
        m = dict(consts)
        m["x"] = np.ascontiguousarray(x[i * NLOC:(i + 1) * NLOC])
        in_maps.append(m)
    nc = _get_nc()
    last_err = None
    for attempt in range(3):
        try:
            res = run_bass_kernel_spmd(nc, in_maps, core_ids=list(range(NCORES)),
                                       trace=trace)
            out = np.concatenate([r["out"] for r in res.results], axis=0)
            return out, res
        except Exception as e:  # transient device wedge: back off and retry
            last_err = e
            time.sleep(10 * (attempt + 1))
    raise last_err


def kernel(**inputs) -> np.ndarray:
    out, _ = _run(inputs, trace=False)
    return out


def kernel_traced(**inputs):
    out, res = _run(inputs, trace=True)
    return out, res
